# revision 3
# baseline (speedup 1.0000x reference)
"""GroupGAT kernel for Trainium2 (Bass/Tile), 8-core data-parallel. v6.

v5 math (host prescale/premask, bf16, 44 node slots, attn_rep broadcast on
ACT, one-TT prodw, varying-stationary PE accumulation), restructured as an
explicit 3-stage software pipeline so the in-order engine queues don't
head-of-line block: stage A (DMA + dot-folds) of tile i issues before stage B
(softmax + prodw) of tile i-1, before stage C (PE accumulation + finals +
elu + store) of tile i-2.

  A: dma hv; e_raw[b,q] = sum_d hv[b,q,d]        (DVE folds + reduce)
  B: e=lrelu(e_raw+s1); attn_rep=exp(e)/den bcast; prodw=attn_rep*hv
     (ACT bias / exp+accum / bcast-scale-copy; DVE lrelu, recip, one 2x TT)
  C: x^T = h0^T + sum_q prodw^T (PE, varying stationary + fixed identity);
     out = elu(x^T.T @ W');  store.
"""

import contextlib

import numpy as np

import concourse.bass as bass
import concourse.bacc as bacc
import concourse.mybir as mybir
from concourse import tile
from concourse.bass_utils import run_bass_kernel_spmd

N_CORES = 8
B = 16384
NUM_NODE = 41
NA = 20
NO = 20
D = 128
NN = 44
B_SHARD = B // N_CORES
P = 128
MASK_FILL = -1e9 / 128.0

F32 = mybir.dt.float32
BF16 = mybir.dt.bfloat16
NPBF16 = mybir.dt.np(BF16)
AL = mybir.AluOpType
AF = mybir.ActivationFunctionType
AX = mybir.AxisListType


def build_nc(b_shard=B_SHARD, repeats=1):
    n_tiles = b_shard // P
    nc = bacc.Bacc("TRN2", target_bir_lowering=False, debug=False)

    hv_d = nc.dram_tensor("hv", [b_shard, NN, D], BF16, kind="ExternalInput").ap()
    wcat_d = nc.dram_tensor("wcat", [P, 2 * D], BF16, kind="ExternalInput").ap()
    identb_d = nc.dram_tensor("identb", [P, P], BF16, kind="ExternalInput").ap()
    out_d = nc.dram_tensor("out", [b_shard, D], F32, kind="ExternalOutput").ap()

    with tile.TileContext(nc) as tc:
        with (
            tc.tile_pool(name="const", bufs=1) as cpool,
            tc.tile_pool(name="hin", bufs=4) as hpool,
            tc.tile_pool(name="small", bufs=4) as spool,
            tc.tile_pool(name="workA", bufs=2) as apool,
            tc.tile_pool(name="workB", bufs=3) as bpool,
            tc.tile_pool(name="workC", bufs=2) as dpool,
            tc.tile_pool(name="psum", bufs=2, space=bass.MemorySpace.PSUM) as ppool,
        ):
            wcat = cpool.tile([P, 2 * D], BF16)
            identb = cpool.tile([P, P], BF16)
            nc.sync.dma_start(wcat[:], wcat_d[:])
            nc.sync.dma_start(identb[:], identb_d[:])
            w_a, w_o = wcat[:, 0:D], wcat[:, D : 2 * D]

            rep_ctx = (
                tc.For_i(0, repeats, 1) if repeats > 1 else contextlib.nullcontext()
            )
            with rep_ctx:
                state = {}
                for step in range(n_tiles + 2):
                    if step < n_tiles:
                        _stage_a(nc, step, hv_d, hpool, apool, spool, state)
                    if 1 <= step <= n_tiles:
                        _stage_b(nc, step - 1, spool, bpool, state)
                    if step >= 2:
                        _stage_c(nc, step - 2, out_d, dpool, ppool,
                                 w_a, w_o, identb, state)

    nc.compile()
    return nc


def _stage_a(nc, it, hv_d, hpool, apool, spool, state):
    b0 = it * P
    ht = hpool.tile([P, NN, D], BF16, tag="ht")
    nc.sync.dma_start(ht[:, 0:22, :], hv_d[b0 : b0 + P, 0:22])
    nc.sync.dma_start(ht[:, 22:NN, :], hv_d[b0 : b0 + P, 22:NN])

    t1 = apool.tile([P, NN, 64], BF16, tag="t1")
    t2 = apool.tile([P, NN, 32], BF16, tag="t2")
    t3 = apool.tile([P, NN, 16], BF16, tag="t3")
    e_raw = spool.tile([P, NN], F32, tag="eraw")
    nc.vector.tensor_add(t1[:], ht[:, :, 0:64], ht[:, :, 64:128])
    nc.vector.tensor_add(t2[:], t1[:, :, 0:32], t1[:, :, 32:64])
    nc.vector.tensor_add(t3[:], t2[:, :, 0:16], t2[:, :, 16:32])
    nc.vector.tensor_reduce(e_raw[:], t3[:], axis=AX.X, op=AL.add)
    state[("ht", it)] = ht
    state[("eraw", it)] = e_raw


def _stage_b(nc, it, spool, bpool, state):
    ht = state[("ht", it)]
    e_raw = state.pop(("eraw", it))

    e_b = spool.tile([P, 42], F32, tag="eb")
    nc.scalar.activation(e_b[:, 0:21], e_raw[:, 0:21], AF.Identity,
                         bias=e_raw[:, 42:43])
    nc.scalar.activation(e_b[:, 21:42], e_raw[:, 21:42], AF.Identity,
                         bias=e_raw[:, 43:44])
    e_l = spool.tile([P, 42], F32, tag="el")
    nc.vector.scalar_tensor_tensor(e_l[:], e_b[:], 0.2, e_b[:], AL.mult, AL.max)
    expe = spool.tile([P, 42], F32, tag="expe")
    den = spool.tile([P, 2], F32, tag="den")
    nc.scalar.activation(expe[:, 0:21], e_l[:, 0:21], AF.Exp,
                         accum_out=den[:, 0:1])
    nc.scalar.activation(expe[:, 21:42], e_l[:, 21:42], AF.Exp,
                         accum_out=den[:, 1:2])
    rec = spool.tile([P, 2], F32, tag="rec")
    nc.vector.reciprocal(rec[:], den[:])

    attn_rep = bpool.tile([P, 42, D], BF16, tag="attnrep")
    expe_ba = expe[:, 0:21].rearrange("p (n o) -> p n o", o=1).broadcast_to(
        [P, 21, D])
    expe_bo = expe[:, 21:42].rearrange("p (n o) -> p n o", o=1).broadcast_to(
        [P, 21, D])
    nc.scalar.activation(attn_rep[:, 0:21, :], expe_ba, AF.Copy,
                         scale=rec[:, 0:1])
    nc.scalar.activation(attn_rep[:, 21:42, :], expe_bo, AF.Copy,
                         scale=rec[:, 1:2])

    prodw = bpool.tile([P, 42, D], BF16, tag="prodw")
    nc.vector.tensor_tensor(prodw[:], ht[:, 0:42, :], attn_rep[:], op=AL.mult)
    state[("prodw", it)] = prodw


def _stage_c(nc, it, out_d, dpool, ppool, w_a, w_o, identb, state):
    b0 = it * P
    ht = state.pop(("ht", it))
    prodw = state.pop(("prodw", it))

    xTa_ps = ppool.tile([P, D], F32, tag="xta")
    xTo_ps = ppool.tile([P, D], F32, tag="xto")
    nc.tensor.matmul(xTa_ps[:], ht[:, 0, :], identb[:], start=True, stop=False)
    for n in range(21):
        nc.tensor.matmul(xTa_ps[:], prodw[:, n, :], identb[:],
                         start=False, stop=(n == 20))
    for n in range(21, 42):
        nc.tensor.matmul(xTo_ps[:], prodw[:, n, :], identb[:],
                         start=(n == 21), stop=(n == 41))

    xTa = dpool.tile([P, D], BF16, tag="xTa")
    xTo = dpool.tile([P, D], BF16, tag="xTo")
    nc.scalar.copy(xTa[:], xTa_ps[:])
    nc.scalar.copy(xTo[:], xTo_ps[:])

    mm = ppool.tile([P, D], F32, tag="mm")
    nc.tensor.matmul(mm[:], xTa[:], w_a, start=True, stop=False)
    nc.tensor.matmul(mm[:], xTo[:], w_o, start=False, stop=True)

    t_neg = dpool.tile([P, D], F32, tag="tneg")
    t_exp = dpool.tile([P, D], F32, tag="texp")
    out_t = dpool.tile([P, D], F32, tag="out")
    nc.scalar.activation(t_neg[:], mm[:], AF.Relu, scale=-1.0)
    nc.scalar.activation(t_exp[:], t_neg[:], AF.Exp, scale=-1.0)
    nc.vector.scalar_tensor_tensor(out_t[:], t_exp[:], -1.0, mm[:],
                                   AL.add, AL.max)
    nc.sync.dma_start(out_d[b0 : b0 + P], out_t[:])


_NC_CACHE = {}


def _get_nc(b_shard):
    if b_shard not in _NC_CACHE:
        _NC_CACHE[b_shard] = build_nc(b_shard)
    return _NC_CACHE[b_shard]


def _host_precompute(h, W_ally, W_opp, a_ally, a_opp, mask):
    v1a = (W_ally @ a_ally[:D, 0]).astype(np.float32)
    v2a = (W_ally @ a_ally[D:, 0]).astype(np.float32)
    v1o = (W_opp @ a_opp[:D, 0]).astype(np.float32)
    v2o = (W_opp @ a_opp[D:, 0]).astype(np.float32)

    def clamp(v):
        s = np.sign(v)
        s[s == 0] = 1.0
        return s * np.maximum(np.abs(v), 1e-6)

    v2a_s, v2o_s = clamp(v2a), clamp(v2o)

    Wa_p = (W_ally / v2a_s[:, None]).astype(np.float32)
    Wo_p = (W_opp / v2o_s[:, None]).astype(np.float32)
    wcat = np.ascontiguousarray(
        np.concatenate([Wa_p, Wo_p], axis=1)).astype(NPBF16)

    hb = np.asarray(h, np.float32)
    b = hb.shape[0]
    hv = np.empty((b, NN, D), np.float32)
    h0 = hb[:, 0]
    hv[:, 0] = h0 * v2a_s
    hv[:, 1:21] = hb[:, 1:21] * v2a_s
    hv[:, 21] = h0 * v2o_s
    hv[:, 22:42] = hb[:, 21:41] * v2o_s
    hv[:, 42] = h0 * v1a
    hv[:, 43] = h0 * v1o
    hv[:, 1:21][mask[:, 1:21]] = MASK_FILL
    hv[:, 22:42][mask[:, 21:41]] = MASK_FILL
    hv = hv.astype(NPBF16)

    identb = np.eye(P, dtype=NPBF16)
    return hv, wcat, identb


def make_in_maps(h, W_ally, W_opp, a_ally, a_opp, mask):
    hv, wcat, identb = _host_precompute(h, W_ally, W_opp, a_ally, a_opp, mask)
    in_maps = []
    for c in range(N_CORES):
        s = slice(c * B_SHARD, (c + 1) * B_SHARD)
        in_maps.append({"hv": hv[s], "wcat": wcat, "identb": identb})
    return in_maps


def kernel(h, W_ally, W_opp, a_ally, a_opp, mask, num_ally, num_opp):
    assert int(num_ally) == NA and int(num_opp) == NO
    h = np.asarray(h, dtype=np.float32)
    mask = np.asarray(mask)
    W_ally = np.asarray(W_ally, dtype=np.float32)
    W_opp = np.asarray(W_opp, dtype=np.float32)
    a_ally = np.asarray(a_ally, dtype=np.float32)
    a_opp = np.asarray(a_opp, dtype=np.float32)

    in_maps = make_in_maps(h, W_ally, W_opp, a_ally, a_opp, mask)
    nc = _get_nc(B_SHARD)
    res = run_bass_kernel_spmd(nc, in_maps, core_ids=list(range(N_CORES)))
    global LAST_RESULTS
    LAST_RESULTS = res
    return np.concatenate([res.results[c]["out"] for c in range(N_CORES)], axis=0)


LAST_RESULTS = None
